# revision 24
# baseline (speedup 1.0000x reference)
"""Trainium2 Bass kernel for nn_Block_70317204570642 (transformer block).

Sharding: 8 cores = 4 batches x 2 sequence halves (data parallel, zero
collectives). Each core computes the full block for its 1024 query rows;
K/V are recomputed locally from the full 2048-row batch. Causality is a
uniform SPMD program + per-core mask DATA:
  - rank-2 augmented rows on the scores matmul (K=66) inject the
    fully-masked-block -1e9 terms,
  - small triangle tiles (DVE adds) handle the 4 diagonal blocks.

Matmuls run in float32r (1 cyc/row on TRN2 for N>=256, ~1.6e-4 rel err).
"""
import sys
import os

sys.path.insert(0, '/opt/trn_rl_repo')

import numpy as np
from contextlib import ExitStack

import concourse.bass as bass
import concourse.bacc as bacc
import concourse.tile as tile
import concourse.mybir as mybir
from concourse.bass_utils import run_bass_kernel_spmd
from concourse.masks import make_identity

F32 = mybir.dt.float32
F32R = mybir.dt.float32r
AF = mybir.ActivationFunctionType

T = 2048       # kv rows per batch
TQ = 1024      # query rows per core
C = 1024       # embed dim
H = 16         # heads
D = 64         # head size
FF = 4096      # ffn hidden
EPS = 1e-5
NKT = 16
NQT = 8
NFC = 8
NEG = -1.0e9
GELU_FUNC = (AF.Tanh if os.environ.get("KSIM_GELU_TANH") else AF.Gelu)
import ml_dtypes
MM_BF16 = bool(os.environ.get("KMM_BF16"))
MMDT = mybir.dt.bfloat16 if MM_BF16 else mybir.dt.float32r
MMNP = ml_dtypes.bfloat16 if MM_BF16 else np.float32


def r(ap):
    return ap


def _layer_norm_inplace(nc, stat_pool, xt, eps_sb):
    """Row-major LN of one [128, C] fp32 tile, normalized in place."""
    stats = stat_pool.tile([128, 2, 6], F32, tag="lnstats", name="lnstats")
    xv = xt[:].rearrange("p (a b) -> p a b", a=2)
    for sg in range(2):
        nc.vector.bn_stats(out=stats[:, sg, :], in_=xv[:, sg, :])
    mv = stat_pool.tile([128, 2], F32, tag="lnmv", name="lnmv")
    nc.vector.bn_aggr(out=mv[:], in_=stats[:])
    rstd = stat_pool.tile([128, 1], F32, tag="lnrstd", name="lnrstd")
    nc.scalar.activation(out=rstd[:], in_=mv[:, 1:2], func=AF.Sqrt,
                         bias=eps_sb[:], scale=1.0)
    nc.vector.reciprocal(out=rstd[:], in_=rstd[:])
    nc.vector.tensor_scalar(
        out=xt[:], in0=xt[:], scalar1=mv[:, 0:1], scalar2=rstd[:],
        op0=mybir.AluOpType.subtract, op1=mybir.AluOpType.mult)


def _stream_ln_transpose(nc, tc, stat_pool, ps_pool, phA, ident, eps_sb,
                         src_ap, n_tiles, dstT):
    """DMA row tiles, LN in place, PE-transpose into dstT chunk tiles."""
    for rg in range(0, n_tiles, 4):
        gn = min(4, n_tiles - rg)
        tiles = []
        for g in range(gn):
            t_ = phA.tile([128, C], F32, tag="xrow", name="xrow")
            i = rg + g
            nc.sync.dma_start(t_[:], src_ap[i * 128:(i + 1) * 128, :])
            _layer_norm_inplace(nc, stat_pool, t_, eps_sb)
            tiles.append(t_)
        for fc in range(NFC):
            ps = ps_pool.tile([128, 512], F32, tag="tps", name="tps")
            for g in range(gn):
                nc.tensor.transpose(
                    ps[:, g * 128:(g + 1) * 128],
                    tiles[g][:, fc * 128:(fc + 1) * 128], ident)
            nc.vector.tensor_copy(
                dstT[fc][:, rg * 128:rg * 128 + gn * 128], ps[:, 0:gn * 128])


def build_program():
    nc = bacc.Bacc("TRN2", target_bir_lowering=False, debug=False,
                   num_devices=8)

    # ---- DRAM I/O ----
    xkv = nc.dram_tensor("xkv", [T, C], F32, kind="ExternalInput").ap()
    xq = nc.dram_tensor("xq", [TQ, C], F32, kind="ExternalInput").ap()
    wqt = nc.dram_tensor("wqt", [C, C], MMDT, kind="ExternalInput").ap()
    wkt = nc.dram_tensor("wkt", [C, C], MMDT, kind="ExternalInput").ap()
    wvt = nc.dram_tensor("wvt", [C, C], MMDT, kind="ExternalInput").ap()
    wot = nc.dram_tensor("wot", [C, C], MMDT, kind="ExternalInput").ap()
    w1 = nc.dram_tensor("w1", [C, FF], MMDT, kind="ExternalInput").ap()
    w2 = nc.dram_tensor("w2", [FF, C], MMDT, kind="ExternalInput").ap()
    bq = nc.dram_tensor("bq", [C, 1], F32, kind="ExternalInput").ap()
    bk = nc.dram_tensor("bk", [C, 1], F32, kind="ExternalInput").ap()
    bvb = nc.dram_tensor("bvb", [128, C], F32, kind="ExternalInput").ap()
    b1 = nc.dram_tensor("b1", [FF, 1], F32, kind="ExternalInput").ap()
    b2b = nc.dram_tensor("b2b", [128, C], F32, kind="ExternalInput").ap()
    kaug = nc.dram_tensor("kaug", [2, T], MMDT, kind="ExternalInput").ap()
    qaug = nc.dram_tensor("qaug", [2, TQ], MMDT, kind="ExternalInput").ap()
    mtri = nc.dram_tensor("mtri", [2, 8, 128, 512], F32,
                          kind="ExternalInput").ap()
    y_out = nc.dram_tensor("y", [TQ, C], F32, kind="ExternalOutput").ap()

    with tile.TileContext(nc) as tc, ExitStack() as ctx:
        const_pool = ctx.enter_context(tc.tile_pool(name="const", bufs=1))
        stat_pool = ctx.enter_context(tc.tile_pool(name="stats", bufs=16))
        ident = const_pool.tile([128, 128], F32)
        make_identity(nc, ident[:])
        ones_f = const_pool.tile([128, 64], F32)
        nc.gpsimd.memset(ones_f[:], 1.0)
        ones_sb = const_pool.tile([1, 64], MMDT)
        nc.vector.tensor_copy(ones_sb[:], ones_f[0:1, :])
        eps_sb = const_pool.tile([128, 1], F32)
        nc.gpsimd.memset(eps_sb[:], EPS)

        dram = ctx.enter_context(tc.tile_pool(name="dram", bufs=1,
                                              space="DRAM"))
        qt_s = dram.tile([C, TQ], MMDT, name="qt_s")
        kt_s = dram.tile([C, T], MMDT, name="kt_s")
        v_s = dram.tile([H, T, D], MMDT, name="v_s")

        # ========== Phase A: LN1 + transpose (streamed) ==========
        with tc.tile_pool(name="xhatT", bufs=1) as xhT_pool:
            xhkvT = [xhT_pool.tile([128, T], MMDT, tag=f"xhkvT{f}",
                                   name=f"xhkvT{f}") for f in range(NFC)]
            xhqT = [xhT_pool.tile([128, TQ], MMDT, tag=f"xhqT{f}",
                                  name=f"xhqT{f}") for f in range(NFC)]
            with tc.tile_pool(name="phA", bufs=12) as phA, \
                 tc.tile_pool(name="ps_tA", bufs=3, space="PSUM") as ps_tA:
                _stream_ln_transpose(nc, tc, stat_pool, ps_tA, phA, ident[:],
                                     eps_sb, xkv, NKT, xhkvT)
                _stream_ln_transpose(nc, tc, stat_pool, ps_tA, phA, ident[:],
                                     eps_sb, xq, NQT, xhqT)

            # ========== Phase C: QKV projections (bounced to DRAM) ==========
            with tc.tile_pool(name="wq", bufs=1) as wpool, \
                 tc.tile_pool(name="bias", bufs=1) as bpool, \
                 tc.tile_pool(name="stg", bufs=6) as stg, \
                 tc.tile_pool(name="ps_C", bufs=4, space="PSUM") as ps_C:
                # q^T
                wq_sb = []
                for kc in range(NFC):
                    wt = wpool.tile([128, C], MMDT, tag=f"wq{kc}",
                                    name=f"wqsb{kc}")
                    nc.sync.dma_start(wt[:], wqt[kc * 128:(kc + 1) * 128, :])
                    wq_sb.append(wt)
                bq_sb = bpool.tile([128, NFC], F32, tag="bq")
                nc.sync.dma_start(
                    bq_sb[:], bq.rearrange("(m p) o -> p (m o)", p=128))
                for mc in range(NFC):
                    for nb in range(TQ // 512):
                        ps = ps_C.tile([128, 512], F32, tag="psqkv",
                                         name="psq")
                        for kc in range(NFC):
                            nc.tensor.matmul(
                                ps[:],
                                r(wq_sb[kc][:, mc * 128:(mc + 1) * 128]),
                                r(xhqT[kc][:, nb * 512:(nb + 1) * 512]),
                                start=(kc == 0), stop=(kc == NFC - 1))
                        st = stg.tile([128, 512], MMDT, tag="stgq", name="stq")
                        nc.scalar.activation(st[:], ps[:], AF.Identity,
                                             bias=bq_sb[:, mc:mc + 1])
                        nc.sync.dma_start(
                            qt_s[mc * 128:(mc + 1) * 128,
                                      nb * 512:(nb + 1) * 512], st[:])
                # k^T
                wk_sb = []
                for kc in range(NFC):
                    wt = wpool.tile([128, C], MMDT, tag=f"wq{kc}",
                                    name=f"wksb{kc}")
                    nc.sync.dma_start(wt[:], wkt[kc * 128:(kc + 1) * 128, :])
                    wk_sb.append(wt)
                bk_sb = bpool.tile([128, NFC], F32, tag="bk")
                nc.sync.dma_start(
                    bk_sb[:], bk.rearrange("(m p) o -> p (m o)", p=128))
                for mc in range(NFC):
                    for nb in range(T // 512):
                        ps = ps_C.tile([128, 512], F32, tag="psqkv",
                                         name="psk")
                        for kc in range(NFC):
                            nc.tensor.matmul(
                                ps[:],
                                r(wk_sb[kc][:, mc * 128:(mc + 1) * 128]),
                                r(xhkvT[kc][:, nb * 512:(nb + 1) * 512]),
                                start=(kc == 0), stop=(kc == NFC - 1))
                        st = stg.tile([128, 512], MMDT, tag="stgq", name="stk")
                        nc.scalar.activation(st[:], ps[:], AF.Identity,
                                             bias=bk_sb[:, mc:mc + 1])
                        nc.sync.dma_start(
                            kt_s[mc * 128:(mc + 1) * 128,
                                      nb * 512:(nb + 1) * 512], st[:])
                # v (row-major, per-head DRAM layout)
                wv_sb = []
                for kc in range(NFC):
                    wt = wpool.tile([128, C], MMDT, tag=f"wq{kc}",
                                    name=f"wvsb{kc}")
                    nc.sync.dma_start(wt[:], wvt[kc * 128:(kc + 1) * 128, :])
                    wv_sb.append(wt)
                bvb_sb = bpool.tile([128, C], F32, tag="bvb")
                nc.sync.dma_start(bvb_sb[:], bvb)
                v_view = v_s[:].rearrange("(g h) t d -> g h t d", g=2)
                for rc in range(NKT):
                    for jb in range(2):
                        ps = ps_C.tile([128, 512], F32, tag="psqkv",
                                         name="psv")
                        for kc in range(NFC):
                            nc.tensor.matmul(
                                ps[:],
                                r(xhkvT[kc][:, rc * 128:(rc + 1) * 128]),
                                r(wv_sb[kc][:, jb * 512:(jb + 1) * 512]),
                                start=(kc == 0), stop=(kc == NFC - 1))
                        st = stg.tile([128, 512], MMDT, tag="stgq", name="stv")
                        nc.vector.tensor_add(
                            st[:], ps[:], bvb_sb[:, jb * 512:(jb + 1) * 512])
                        dst = v_view[jb, :, rc * 128:(rc + 1) * 128,
                                     :].rearrange("h p d -> p h d")
                        nc.sync.dma_start(
                            dst, st[:].rearrange("p (h d) -> p h d", d=D))

        # ========== Phase D: attention ==========
        with tc.tile_pool(name="y1p", bufs=1) as y1p:
            y1 = [y1p.tile([128, C], F32, tag=f"y1_{i}", name=f"y1_{i}")
                  for i in range(NQT)]
            with tc.tile_pool(name="catT", bufs=1) as cat_pool:
                catT = [cat_pool.tile([128, TQ], MMDT, tag=f"catT{f}",
                                      name=f"catT{f}") for f in range(NFC)]
                with tc.tile_pool(name="attn_in", bufs=3) as ain, \
                     tc.tile_pool(name="mask", bufs=1) as mpool, \
                     tc.tile_pool(name="epool", bufs=4) as epool, \
                     tc.tile_pool(name="nrm", bufs=4) as nrm, \
                     tc.tile_pool(name="ps_s", bufs=2, space="PSUM") as pssp, \
                     tc.tile_pool(name="ps_o", bufs=2, space="PSUM") as psop:
                    mtri_sb = mpool.tile([128, 2, 8, 512], F32)
                    nc.sync.dma_start(
                        mtri_sb[:], mtri.rearrange("c s p n -> p c s n"))
                    for h in range(H):
                        q_t = ain.tile([66, TQ], MMDT, tag="q_t", name="q_t")
                        nc.sync.dma_start(
                            q_t[0:64, :], qt_s[h * 64:(h + 1) * 64, :])
                        nc.sync.dma_start(q_t[64:66, :], qaug)
                        k_t = ain.tile([66, T], MMDT, tag="k_t", name="k_t")
                        nc.sync.dma_start(
                            k_t[0:64, :], kt_s[h * 64:(h + 1) * 64, :])
                        nc.sync.dma_start(k_t[64:66, :], kaug)
                        v_t = ain.tile([128, NKT, D + 1], MMDT, tag="v_t",
                                       name="v_t")
                        nc.vector.tensor_copy(
                            v_t[:, :, D:D + 1],
                            ones_f[:, 0:16].rearrange("p (a b) -> p a b", b=1))
                        nc.sync.dma_start(
                            v_t[:, :, 0:D],
                            v_s[h].rearrange("(rc p) d -> p rc d", p=128))
                        NKB = {0: 12, 1: 16}
                        ps_o = {}
                        for c_ in range(2):
                            ps_o[c_] = psop.tile([65, 512], F32, tag="ps_o",
                                                 name="ps_o")
                        sched = []
                        for g in range(8):
                            for c_ in range(2):
                                if g < NKB[c_] // 2:
                                    sched.append((c_, g))
                        for c_, g in sched:
                            nkb = NKB[c_]
                            ps_s = pssp.tile([128, 1024], F32, tag="ps_s",
                                             name="ps_s")
                            for gi in range(2):
                                kb = g * 2 + gi
                                nc.tensor.matmul(
                                    ps_s[:, gi * 512:(gi + 1) * 512],
                                    r(k_t[:, kb * 128:(kb + 1) * 128]),
                                    r(q_t[:, c_ * 512:(c_ + 1) * 512]),
                                    start=True, stop=True)
                                s = -1
                                if 4 * c_ <= kb < 4 * c_ + 4:
                                    s = kb - 4 * c_
                                elif 8 + 4 * c_ <= kb < 12 + 4 * c_:
                                    s = kb - 8 - 4 * c_ + 4
                                if s >= 0:
                                    nc.vector.tensor_add(
                                        ps_s[:, gi * 512:(gi + 1) * 512],
                                        ps_s[:, gi * 512:(gi + 1) * 512],
                                        mtri_sb[:, c_, s, :])
                            e_t = epool.tile([128, 1024], MMDT, tag="e_t",
                                             name="e_t")
                            nc.scalar.activation(e_t[:], ps_s[:], AF.Exp,
                                                 scale=float(D) ** -0.5)
                            for gi in range(2):
                                kb = g * 2 + gi
                                nc.tensor.matmul(
                                    ps_o[c_][:],
                                    r(v_t[:, kb, :]),
                                    r(e_t[:, gi * 512:(gi + 1) * 512]),
                                    start=(kb == 0), stop=(kb == nkb - 1))
                        for c_ in range(2):
                            den = nrm.tile([1, 512], F32, tag="den",
                                           name="den")
                            nc.vector.tensor_copy(den[:], ps_o[c_][64:65, :])
                            rden = nrm.tile([1, 512], F32, tag="rden",
                                            name="rden")
                            nc.vector.reciprocal_approx_fast(out=rden[:],
                                                             in_=den[:])
                            rden_r = nrm.tile([1, 512], MMDT, tag="rden_r",
                                              name="rden_r")
                            nc.vector.tensor_copy(rden_r[:], rden[:])
                            ps_b = psop.tile([64, 512], F32, tag="ps_b",
                                             name="ps_b", bufs=1)
                            nc.tensor.matmul(ps_b[:], r(ones_sb[:]),
                                             r(rden_r[:]), start=True,
                                             stop=True)
                            bc_sb = nrm.tile([64, 512], F32, tag="bc_sb",
                                             name="bc_sb")
                            nc.vector.tensor_copy(bc_sb[:], ps_b[:])
                            tmp = nrm.tile([64, 512], MMDT, tag="nrmtmp",
                                           name="nrmtmp")
                            nc.vector.tensor_mul(tmp[:], ps_o[c_][0:64, :],
                                                 bc_sb[:])
                            nc.sync.dma_start(
                                catT[h // 2][(h % 2) * 64:(h % 2) * 64 + 64,
                                             c_ * 512:(c_ + 1) * 512],
                                tmp[:])

                # ========== Phase E: Wo + residual ==========
                with tc.tile_pool(name="phE", bufs=1) as phE, \
                     tc.tile_pool(name="ps_E", bufs=4, space="PSUM") as ps_E:
                    wo_sb = []
                    for fc in range(NFC):
                        wt = phE.tile([128, C], MMDT, tag=f"wo{fc}",
                                      name=f"wo{fc}")
                        nc.sync.dma_start(wt[:],
                                          wot[fc * 128:(fc + 1) * 128, :])
                        wo_sb.append(wt)
                    xq_sb = []
                    for i in range(NQT):
                        t_ = phE.tile([128, C], F32, tag=f"xq2_{i}",
                                      name=f"xq2_{i}")
                        nc.sync.dma_start(t_[:], xq[i * 128:(i + 1) * 128, :])
                        xq_sb.append(t_)
                    for qi in range(NQT):
                        for jb in range(2):
                            ps = ps_E.tile([128, 512], F32, tag="psqkv",
                                             name="pso")
                            for fc in range(NFC):
                                nc.tensor.matmul(
                                    ps[:],
                                    r(catT[fc][:, qi * 128:(qi + 1) * 128]),
                                    r(wo_sb[fc][:, jb * 512:(jb + 1) * 512]),
                                    start=(fc == 0), stop=(fc == NFC - 1))
                            nc.vector.tensor_add(
                                y1[qi][:, jb * 512:(jb + 1) * 512], ps[:],
                                xq_sb[qi][:, jb * 512:(jb + 1) * 512])

            # ========== Phase F: LN2 + FFN ==========
            with tc.tile_pool(name="phF", bufs=1) as phF, \
                 tc.tile_pool(name="ps_F", bufs=4, space="PSUM") as ps_F, \
                 tc.tile_pool(name="ps_tF", bufs=2, space="PSUM") as ps_tF:
                xh2T = [phF.tile([128, TQ], MMDT, tag=f"xh2T{f}",
                                 name=f"xh2T{f}") for f in range(NFC)]
                with tc.tile_pool(name="phF2", bufs=8) as phF2:
                    for rg in range(0, NQT, 4):
                        tiles = []
                        for g in range(4):
                            t_ = phF2.tile([128, C], F32, tag="h2row",
                                           name="h2row")
                            nc.vector.tensor_copy(t_[:], y1[rg + g][:])
                            _layer_norm_inplace(nc, stat_pool, t_, eps_sb)
                            tiles.append(t_)
                        for fc in range(NFC):
                            ps = ps_tF.tile([128, 512], F32, tag="tps",
                                           name="tps")
                            for g in range(4):
                                nc.tensor.transpose(
                                    ps[:, g * 128:(g + 1) * 128],
                                    tiles[g][:, fc * 128:(fc + 1) * 128],
                                    ident[:])
                            nc.vector.tensor_copy(
                                xh2T[fc][:, rg * 128:rg * 128 + 512],
                                ps[:])

                b1_sb = phF.tile([128, FF // 128], F32, tag="b1")
                nc.sync.dma_start(
                    b1_sb[:], b1.rearrange("(m p) o -> p (m o)", p=128))
                b2b_sb = phF.tile([128, C], F32, tag="b2b")
                nc.sync.dma_start(b2b_sb[:], b2b)
                y_acc = [phF.tile([128, C], F32, tag=f"yacc{i}",
                                  name=f"yacc{i}") for i in range(NQT)]
                for qi in range(NQT):
                    nc.vector.tensor_add(y_acc[qi][:], y1[qi][:], b2b_sb[:])
                with tc.tile_pool(name="w1p", bufs=3) as w1p, \
                     tc.tile_pool(name="w2p", bufs=10) as w2p, \
                     tc.tile_pool(name="ffp", bufs=1) as ffp:
                    for stripe in range(4):
                        ff_sb = [ffp.tile([128, TQ], MMDT, tag=f"ff{m}",
                                          name=f"ff{m}") for m in range(8)]
                        w1v = w1.rearrange("(kc p) m -> p kc m", p=128)
                        for m in range(8):
                            mc = stripe * 8 + m
                            w1t = w1p.tile([128, NFC, 128], MMDT, tag="w1t",
                                           name="w1t")
                            nc.sync.dma_start(
                                w1t[:], w1v[:, :, mc * 128:(mc + 1) * 128])
                            w1_sb = [w1t[:, kc, :] for kc in range(NFC)]
                            for nb in range(2):
                                ps = ps_F.tile([128, 512], F32, tag="psqkv",
                                                 name="psf1")
                                for kc in range(NFC):
                                    nc.tensor.matmul(
                                        ps[:],
                                        w1_sb[kc],
                                        r(xh2T[kc][:, nb * 512:
                                                   (nb + 1) * 512]),
                                        start=(kc == 0), stop=(kc == NFC - 1))
                                nc.scalar.activation(
                                    ff_sb[m][:, nb * 512:(nb + 1) * 512],
                                    ps[:], GELU_FUNC,
                                    bias=b1_sb[:, mc:mc + 1])
                        w2_sb = []
                        for m in range(8):
                            mc = stripe * 8 + m
                            wt = w2p.tile([128, C], MMDT, tag="w2t",
                                          name="w2t")
                            nc.sync.dma_start(
                                wt[:], w2[mc * 128:(mc + 1) * 128, :])
                            w2_sb.append(wt)
                        for qi in range(NQT):
                            for jb in range(2):
                                ps = ps_F.tile([128, 512], F32, tag="psqkv",
                                                 name="psf2")
                                for m in range(8):
                                    nc.tensor.matmul(
                                        ps[:],
                                        r(ff_sb[m][:, qi * 128:
                                                   (qi + 1) * 128]),
                                        r(w2_sb[m][:, jb * 512:
                                                   (jb + 1) * 512]),
                                        start=(m == 0), stop=(m == 7))
                                nc.vector.tensor_add(
                                    y_acc[qi][:, jb * 512:(jb + 1) * 512],
                                    y_acc[qi][:, jb * 512:(jb + 1) * 512],
                                    ps[:])
                for qi in range(NQT):
                    nc.sync.dma_start(y_out[qi * 128:(qi + 1) * 128, :],
                                      y_acc[qi][:])

    nc.compile()
    return nc


_NC_CACHE = None


def _get_nc():
    global _NC_CACHE
    if _NC_CACHE is None:
        _NC_CACHE = build_program()
    return _NC_CACHE


def _prep_in_maps(x, Wq, Wk, Wv, Wo, W1, b1, W2, b2, g1, be1, g2, be2):
    x = np.asarray(x, dtype=np.float32)
    Wq = np.asarray(Wq, np.float32); Wk = np.asarray(Wk, np.float32)
    Wv = np.asarray(Wv, np.float32); Wo = np.asarray(Wo, np.float32)
    W1 = np.asarray(W1, np.float32); W2 = np.asarray(W2, np.float32)
    b1 = np.asarray(b1, np.float32); b2 = np.asarray(b2, np.float32)
    g1 = np.asarray(g1, np.float32); be1 = np.asarray(be1, np.float32)
    g2 = np.asarray(g2, np.float32); be2 = np.asarray(be2, np.float32)

    # host-side weight folding (LN affine into projections)
    wqt = np.ascontiguousarray((Wq * g1[None, :]).T)     # [c, j]
    wkt = np.ascontiguousarray((Wk * g1[None, :]).T)
    wvt = np.ascontiguousarray((Wv * g1[None, :]).T)
    wot = np.ascontiguousarray(Wo.T)                      # [f, j]
    w1e = np.ascontiguousarray(W1 * g2[:, None])          # [c, m]
    bq = (be1 @ Wq.T).reshape(C, 1)
    bk = (be1 @ Wk.T).reshape(C, 1)
    bv = (be1 @ Wv.T)                                     # [C]
    bvb = np.ascontiguousarray(np.broadcast_to(bv[None, :], (128, C)))
    b1e = (b1 + be2 @ W1).reshape(FF, 1)
    b2b = np.ascontiguousarray(np.broadcast_to(b2[None, :], (128, C)))

    qaug = np.zeros((2, TQ), np.float32)
    qaug[0, :512] = 1.0
    qaug[1, 512:] = 1.0

    tri = np.zeros((4, 128, 512), np.float32)
    ki = np.arange(128)[:, None]
    qi = np.arange(512)[None, :]
    for rr in range(4):
        tri[rr] = np.where(128 * rr + ki <= qi, 0.0, NEG)

    in_maps = []
    for core in range(8):
        b_, h_ = core // 2, core % 2
        kaug = np.zeros((2, T), np.float32)
        kk = np.arange(T)
        kaug[0, kk >= 1024 * h_ + 512] = NEG
        kaug[1, kk >= 1024 * h_ + 1024] = NEG
        mtri = np.zeros((2, 8, 128, 512), np.float32)
        for c_ in range(2):
            for s in range(8):
                if h_ == 0 and s < 4:
                    mtri[c_, s] = tri[s]
                elif h_ == 1 and s >= 4:
                    mtri[c_, s] = tri[s - 4]
        in_maps.append({
            "xkv": np.ascontiguousarray(x[b_]),
            "xq": np.ascontiguousarray(x[b_, h_ * TQ:(h_ + 1) * TQ]),
            "wqt": wqt.astype(MMNP), "wkt": wkt.astype(MMNP),
            "wvt": wvt.astype(MMNP), "wot": wot.astype(MMNP),
            "w1": w1e.astype(MMNP), "w2": W2.astype(MMNP),
            "bq": bq, "bk": bk, "bvb": bvb, "b1": b1e, "b2b": b2b,
            "kaug": kaug.astype(MMNP), "qaug": qaug.astype(MMNP),
            "mtri": mtri,
        })
    return in_maps


def _gather(res):
    y = np.empty((4, T, C), np.float32)
    for core in range(8):
        b_, h_ = core // 2, core % 2
        y[b_, h_ * TQ:(h_ + 1) * TQ] = res.results[core]["y"]
    return y


def kernel(**inputs):
    in_maps = _prep_in_maps(**inputs)
    nc = _get_nc()
    res = run_bass_kernel_spmd(nc, in_maps, list(range(8)))
    return _gather(res)


def kernel_traced(**inputs):
    import tempfile
    in_maps = _prep_in_maps(**inputs)
    nc = _get_nc()
    res = run_bass_kernel_spmd(nc, in_maps, list(range(8)), trace=True,
                               tmpdir=tempfile.mkdtemp(prefix="ktrace_"))
    return _gather(res), res.exec_time_ns


# revision 25
# speedup vs baseline: 1.1036x; 1.1036x over previous
"""Trainium2 Bass kernel for nn_Block_70317204570642 (transformer block).

Sharding: 8 cores = 4 batches x 2 sequence halves (data parallel, zero
collectives). Each core computes the full block for its 1024 query rows;
K/V are recomputed locally from the full 2048-row batch. Causality is a
uniform SPMD program + per-core mask DATA:
  - rank-2 augmented rows on the scores matmul (K=66) inject the
    fully-masked-block -1e9 terms,
  - small triangle tiles (DVE adds) handle the 4 diagonal blocks.

Matmuls run in float32r (1 cyc/row on TRN2 for N>=256, ~1.6e-4 rel err).
"""
import sys
import os

sys.path.insert(0, '/opt/trn_rl_repo')

import numpy as np
from contextlib import ExitStack

import concourse.bass as bass
import concourse.bacc as bacc
import concourse.tile as tile
import concourse.mybir as mybir
from concourse.bass_utils import run_bass_kernel_spmd
from concourse.masks import make_identity

F32 = mybir.dt.float32
F32R = mybir.dt.float32r
AF = mybir.ActivationFunctionType

T = 2048       # kv rows per batch
TQ = 1024      # query rows per core
C = 1024       # embed dim
H = 16         # heads
D = 64         # head size
FF = 4096      # ffn hidden
EPS = 1e-5
NKT = 16
NQT = 8
NFC = 8
NEG = -1.0e9
GELU_FUNC = (AF.Tanh if os.environ.get("KSIM_GELU_TANH") else AF.Gelu)
import ml_dtypes
MM_BF16 = bool(os.environ.get("KMM_BF16"))
MMDT = mybir.dt.bfloat16 if MM_BF16 else mybir.dt.float32r
MMNP = ml_dtypes.bfloat16 if MM_BF16 else np.float32


def r(ap):
    return ap


def _layer_norm_inplace(nc, stat_pool, xt, eps_sb):
    """Row-major LN of one [128, C] fp32 tile, normalized in place."""
    stats = stat_pool.tile([128, 2, 6], F32, tag="lnstats", name="lnstats")
    xv = xt[:].rearrange("p (a b) -> p a b", a=2)
    for sg in range(2):
        nc.vector.bn_stats(out=stats[:, sg, :], in_=xv[:, sg, :])
    mv = stat_pool.tile([128, 2], F32, tag="lnmv", name="lnmv")
    nc.vector.bn_aggr(out=mv[:], in_=stats[:])
    rstd = stat_pool.tile([128, 1], F32, tag="lnrstd", name="lnrstd")
    nc.scalar.activation(out=rstd[:], in_=mv[:, 1:2], func=AF.Sqrt,
                         bias=eps_sb[:], scale=1.0)
    nc.vector.reciprocal(out=rstd[:], in_=rstd[:])
    nc.vector.tensor_scalar(
        out=xt[:], in0=xt[:], scalar1=mv[:, 0:1], scalar2=rstd[:],
        op0=mybir.AluOpType.subtract, op1=mybir.AluOpType.mult)


def _stream_ln_transpose(nc, tc, stat_pool, ps_pool, phA, ident, eps_sb,
                         src_ap, n_tiles, dstT):
    """DMA row tiles, LN in place, PE-transpose into dstT chunk tiles."""
    for rg in range(0, n_tiles, 4):
        gn = min(4, n_tiles - rg)
        tiles = []
        for g in range(gn):
            t_ = phA.tile([128, C], F32, tag="xrow", name="xrow")
            i = rg + g
            nc.sync.dma_start(t_[:], src_ap[i * 128:(i + 1) * 128, :])
            _layer_norm_inplace(nc, stat_pool, t_, eps_sb)
            tiles.append(t_)
        for fc in range(NFC):
            ps = ps_pool.tile([128, 512], F32, tag="tps", name="tps")
            for g in range(gn):
                nc.tensor.transpose(
                    ps[:, g * 128:(g + 1) * 128],
                    tiles[g][:, fc * 128:(fc + 1) * 128], ident)
            nc.vector.tensor_copy(
                dstT[fc][:, rg * 128:rg * 128 + gn * 128], ps[:, 0:gn * 128])


def build_program():
    nc = bacc.Bacc("TRN2", target_bir_lowering=False, debug=False,
                   num_devices=8)

    # ---- DRAM I/O ----
    xkv = nc.dram_tensor("xkv", [T, C], F32, kind="ExternalInput").ap()
    xq = nc.dram_tensor("xq", [TQ, C], F32, kind="ExternalInput").ap()
    wqt = nc.dram_tensor("wqt", [C, C], MMDT, kind="ExternalInput").ap()
    wkt = nc.dram_tensor("wkt", [C, C], MMDT, kind="ExternalInput").ap()
    wvt = nc.dram_tensor("wvt", [C, C], MMDT, kind="ExternalInput").ap()
    wot = nc.dram_tensor("wot", [C, C], MMDT, kind="ExternalInput").ap()
    w1 = nc.dram_tensor("w1", [C, FF], MMDT, kind="ExternalInput").ap()
    w2 = nc.dram_tensor("w2", [FF, C], MMDT, kind="ExternalInput").ap()
    bq = nc.dram_tensor("bq", [C, 1], F32, kind="ExternalInput").ap()
    bk = nc.dram_tensor("bk", [C, 1], F32, kind="ExternalInput").ap()
    bvb = nc.dram_tensor("bvb", [128, C], F32, kind="ExternalInput").ap()
    b1 = nc.dram_tensor("b1", [FF, 1], F32, kind="ExternalInput").ap()
    b2b = nc.dram_tensor("b2b", [128, C], F32, kind="ExternalInput").ap()
    kaug = nc.dram_tensor("kaug", [2, T], MMDT, kind="ExternalInput").ap()
    qaug = nc.dram_tensor("qaug", [2, TQ], MMDT, kind="ExternalInput").ap()
    mtri = nc.dram_tensor("mtri", [2, 8, 128, 512], F32,
                          kind="ExternalInput").ap()
    y_out = nc.dram_tensor("y", [TQ, C], F32, kind="ExternalOutput").ap()

    with tile.TileContext(nc) as tc, ExitStack() as ctx:
        const_pool = ctx.enter_context(tc.tile_pool(name="const", bufs=1))
        stat_pool = ctx.enter_context(tc.tile_pool(name="stats", bufs=16))
        ident = const_pool.tile([128, 128], F32)
        make_identity(nc, ident[:])
        ones_f = const_pool.tile([128, 64], F32)
        nc.gpsimd.memset(ones_f[:], 1.0)
        ones_sb = const_pool.tile([1, 64], MMDT)
        nc.vector.tensor_copy(ones_sb[:], ones_f[0:1, :])
        eps_sb = const_pool.tile([128, 1], F32)
        nc.gpsimd.memset(eps_sb[:], EPS)

        dram = ctx.enter_context(tc.tile_pool(name="dram", bufs=1,
                                              space="DRAM"))
        qt_s = dram.tile([C, TQ], MMDT, name="qt_s")
        kt_s = dram.tile([C, T], MMDT, name="kt_s")
        v_s = dram.tile([H, T, D], MMDT, name="v_s")

        # ========== Phase A: LN1 + transpose (streamed) ==========
        with tc.tile_pool(name="xhatT", bufs=1) as xhT_pool:
            xhkvT = [xhT_pool.tile([128, T], MMDT, tag=f"xhkvT{f}",
                                   name=f"xhkvT{f}") for f in range(NFC)]
            xhqT = [xhT_pool.tile([128, TQ], MMDT, tag=f"xhqT{f}",
                                  name=f"xhqT{f}") for f in range(NFC)]
            with tc.tile_pool(name="phA", bufs=12) as phA, \
                 tc.tile_pool(name="ps_tA", bufs=3, space="PSUM") as ps_tA:
                _stream_ln_transpose(nc, tc, stat_pool, ps_tA, phA, ident[:],
                                     eps_sb, xkv, NKT, xhkvT)
                _stream_ln_transpose(nc, tc, stat_pool, ps_tA, phA, ident[:],
                                     eps_sb, xq, NQT, xhqT)

            # ========== Phase C: QKV projections (bounced to DRAM) ==========
            with tc.tile_pool(name="wq", bufs=1) as wpool, \
                 tc.tile_pool(name="bias", bufs=1) as bpool, \
                 tc.tile_pool(name="stg", bufs=6) as stg, \
                 tc.tile_pool(name="ps_C", bufs=4, space="PSUM") as ps_C:
                # q^T
                wq_sb = []
                for kc in range(NFC):
                    wt = wpool.tile([128, C], MMDT, tag=f"wq{kc}",
                                    name=f"wqsb{kc}")
                    nc.sync.dma_start(wt[:], wqt[kc * 128:(kc + 1) * 128, :])
                    wq_sb.append(wt)
                bq_sb = bpool.tile([128, NFC], F32, tag="bq")
                nc.sync.dma_start(
                    bq_sb[:], bq.rearrange("(m p) o -> p (m o)", p=128))
                for mc in range(NFC):
                    for nb in range(TQ // 512):
                        ps = ps_C.tile([128, 512], F32, tag="psqkv",
                                         name="psq")
                        for kc in range(NFC):
                            nc.tensor.matmul(
                                ps[:],
                                r(wq_sb[kc][:, mc * 128:(mc + 1) * 128]),
                                r(xhqT[kc][:, nb * 512:(nb + 1) * 512]),
                                start=(kc == 0), stop=(kc == NFC - 1))
                        st = stg.tile([128, 512], MMDT, tag="stgq", name="stq")
                        nc.scalar.activation(st[:], ps[:], AF.Identity,
                                             bias=bq_sb[:, mc:mc + 1])
                        nc.sync.dma_start(
                            qt_s[mc * 128:(mc + 1) * 128,
                                      nb * 512:(nb + 1) * 512], st[:])
                # k^T
                wk_sb = []
                for kc in range(NFC):
                    wt = wpool.tile([128, C], MMDT, tag=f"wq{kc}",
                                    name=f"wksb{kc}")
                    nc.sync.dma_start(wt[:], wkt[kc * 128:(kc + 1) * 128, :])
                    wk_sb.append(wt)
                bk_sb = bpool.tile([128, NFC], F32, tag="bk")
                nc.sync.dma_start(
                    bk_sb[:], bk.rearrange("(m p) o -> p (m o)", p=128))
                for mc in range(NFC):
                    for nb in range(T // 512):
                        ps = ps_C.tile([128, 512], F32, tag="psqkv",
                                         name="psk")
                        for kc in range(NFC):
                            nc.tensor.matmul(
                                ps[:],
                                r(wk_sb[kc][:, mc * 128:(mc + 1) * 128]),
                                r(xhkvT[kc][:, nb * 512:(nb + 1) * 512]),
                                start=(kc == 0), stop=(kc == NFC - 1))
                        st = stg.tile([128, 512], MMDT, tag="stgq", name="stk")
                        nc.scalar.activation(st[:], ps[:], AF.Identity,
                                             bias=bk_sb[:, mc:mc + 1])
                        nc.sync.dma_start(
                            kt_s[mc * 128:(mc + 1) * 128,
                                      nb * 512:(nb + 1) * 512], st[:])
                # v (row-major, per-head DRAM layout)
                wv_sb = []
                for kc in range(NFC):
                    wt = wpool.tile([128, C], MMDT, tag=f"wq{kc}",
                                    name=f"wvsb{kc}")
                    nc.sync.dma_start(wt[:], wvt[kc * 128:(kc + 1) * 128, :])
                    wv_sb.append(wt)
                bvb_sb = bpool.tile([128, C], F32, tag="bvb")
                nc.sync.dma_start(bvb_sb[:], bvb)
                v_view = v_s[:].rearrange("(g h) t d -> g h t d", g=2)
                for rc in range(NKT):
                    for jb in range(2):
                        ps = ps_C.tile([128, 512], F32, tag="psqkv",
                                         name="psv")
                        for kc in range(NFC):
                            nc.tensor.matmul(
                                ps[:],
                                r(xhkvT[kc][:, rc * 128:(rc + 1) * 128]),
                                r(wv_sb[kc][:, jb * 512:(jb + 1) * 512]),
                                start=(kc == 0), stop=(kc == NFC - 1))
                        st = stg.tile([128, 512], MMDT, tag="stgq", name="stv")
                        nc.vector.tensor_add(
                            st[:], ps[:], bvb_sb[:, jb * 512:(jb + 1) * 512])
                        dst = v_view[jb, :, rc * 128:(rc + 1) * 128,
                                     :].rearrange("h p d -> p h d")
                        nc.sync.dma_start(
                            dst, st[:].rearrange("p (h d) -> p h d", d=D))

        # ========== Phase D: attention ==========
        with tc.tile_pool(name="y1p", bufs=1) as y1p:
            y1 = [y1p.tile([128, C], F32, tag=f"y1_{i}", name=f"y1_{i}")
                  for i in range(NQT)]
            with tc.tile_pool(name="catT", bufs=1) as cat_pool:
                catT = [cat_pool.tile([128, TQ], MMDT, tag=f"catT{f}",
                                      name=f"catT{f}") for f in range(NFC)]
                with tc.tile_pool(name="attn_in", bufs=3) as ain, \
                     tc.tile_pool(name="mask", bufs=1) as mpool, \
                     tc.tile_pool(name="epool", bufs=4) as epool, \
                     tc.tile_pool(name="nrm", bufs=4) as nrm, \
                     tc.tile_pool(name="ps_s", bufs=2, space="PSUM") as pssp, \
                     tc.tile_pool(name="ps_o", bufs=2, space="PSUM") as psop:
                    mtri_sb = mpool.tile([128, 2, 8, 512], F32)
                    nc.sync.dma_start(
                        mtri_sb[:], mtri.rearrange("c s p n -> p c s n"))
                    for h in range(H):
                        q_t = ain.tile([66, TQ], MMDT, tag="q_t", name="q_t")
                        nc.sync.dma_start(
                            q_t[0:64, :], qt_s[h * 64:(h + 1) * 64, :])
                        nc.sync.dma_start(q_t[64:66, :], qaug)
                        k_t = ain.tile([66, T], MMDT, tag="k_t", name="k_t")
                        nc.sync.dma_start(
                            k_t[0:64, :], kt_s[h * 64:(h + 1) * 64, :])
                        nc.sync.dma_start(k_t[64:66, :], kaug)
                        v_t = ain.tile([128, NKT, D + 1], MMDT, tag="v_t",
                                       name="v_t")
                        nc.vector.tensor_copy(
                            v_t[:, :, D:D + 1],
                            ones_f[:, 0:16].rearrange("p (a b) -> p a b", b=1))
                        nc.sync.dma_start(
                            v_t[:, :, 0:D],
                            v_s[h].rearrange("(rc p) d -> p rc d", p=128))
                        NKB = {0: 12, 1: 16}
                        ps_o = {}
                        for c_ in range(2):
                            ps_o[c_] = psop.tile([65, 512], F32, tag="ps_o",
                                                 name="ps_o")
                        sched = []
                        for g in range(8):
                            for c_ in range(2):
                                if g < NKB[c_] // 2:
                                    sched.append((c_, g))
                        for c_, g in sched:
                            nkb = NKB[c_]
                            ps_s = pssp.tile([128, 1024], F32, tag="ps_s",
                                             name="ps_s")
                            for gi in range(2):
                                kb = g * 2 + gi
                                nc.tensor.matmul(
                                    ps_s[:, gi * 512:(gi + 1) * 512],
                                    r(k_t[:, kb * 128:(kb + 1) * 128]),
                                    r(q_t[:, c_ * 512:(c_ + 1) * 512]),
                                    start=True, stop=True)
                            s0 = -1
                            if 2 * c_ <= g < 2 * c_ + 2:
                                s0 = (g - 2 * c_) * 2
                            elif 4 + 2 * c_ <= g < 6 + 2 * c_:
                                s0 = (g - 4 - 2 * c_) * 2 + 4
                            if s0 >= 0:
                                mv_ = mtri_sb[:, c_, s0:s0 + 2, :].rearrange(
                                    "p s n -> p (s n)")
                                nc.vector.tensor_add(ps_s[:], ps_s[:], mv_)
                            e_t = epool.tile([128, 1024], MMDT, tag="e_t",
                                             name="e_t")
                            nc.scalar.activation(e_t[:], ps_s[:], AF.Exp,
                                                 scale=float(D) ** -0.5)
                            for gi in range(2):
                                kb = g * 2 + gi
                                nc.tensor.matmul(
                                    ps_o[c_][:],
                                    r(v_t[:, kb, :]),
                                    r(e_t[:, gi * 512:(gi + 1) * 512]),
                                    start=(kb == 0), stop=(kb == nkb - 1))
                        for c_ in range(2):
                            den = nrm.tile([1, 512], F32, tag="den",
                                           name="den")
                            nc.vector.tensor_copy(den[:], ps_o[c_][64:65, :])
                            rden = nrm.tile([1, 512], F32, tag="rden",
                                            name="rden")
                            nc.vector.reciprocal_approx_fast(out=rden[:],
                                                             in_=den[:])
                            rden_r = nrm.tile([1, 512], MMDT, tag="rden_r",
                                              name="rden_r")
                            nc.vector.tensor_copy(rden_r[:], rden[:])
                            ps_b = psop.tile([64, 512], F32, tag="ps_b",
                                             name="ps_b", bufs=1)
                            nc.tensor.matmul(ps_b[:], r(ones_sb[:]),
                                             r(rden_r[:]), start=True,
                                             stop=True)
                            bc_sb = nrm.tile([64, 512], F32, tag="bc_sb",
                                             name="bc_sb")
                            nc.vector.tensor_copy(bc_sb[:], ps_b[:])
                            tmp = nrm.tile([64, 512], MMDT, tag="nrmtmp",
                                           name="nrmtmp")
                            nc.vector.tensor_mul(tmp[:], ps_o[c_][0:64, :],
                                                 bc_sb[:])
                            nc.sync.dma_start(
                                catT[h // 2][(h % 2) * 64:(h % 2) * 64 + 64,
                                             c_ * 512:(c_ + 1) * 512],
                                tmp[:])

                # ========== Phase E: Wo + residual ==========
                with tc.tile_pool(name="phE", bufs=1) as phE, \
                     tc.tile_pool(name="ps_E", bufs=4, space="PSUM") as ps_E:
                    wo_sb = []
                    for fc in range(NFC):
                        wt = phE.tile([128, C], MMDT, tag=f"wo{fc}",
                                      name=f"wo{fc}")
                        nc.sync.dma_start(wt[:],
                                          wot[fc * 128:(fc + 1) * 128, :])
                        wo_sb.append(wt)
                    xq_sb = []
                    for i in range(NQT):
                        t_ = phE.tile([128, C], F32, tag=f"xq2_{i}",
                                      name=f"xq2_{i}")
                        nc.sync.dma_start(t_[:], xq[i * 128:(i + 1) * 128, :])
                        xq_sb.append(t_)
                    for qi in range(NQT):
                        for jb in range(2):
                            ps = ps_E.tile([128, 512], F32, tag="psqkv",
                                             name="pso")
                            for fc in range(NFC):
                                nc.tensor.matmul(
                                    ps[:],
                                    r(catT[fc][:, qi * 128:(qi + 1) * 128]),
                                    r(wo_sb[fc][:, jb * 512:(jb + 1) * 512]),
                                    start=(fc == 0), stop=(fc == NFC - 1))
                            nc.vector.tensor_add(
                                y1[qi][:, jb * 512:(jb + 1) * 512], ps[:],
                                xq_sb[qi][:, jb * 512:(jb + 1) * 512])

            # ========== Phase F: LN2 + FFN ==========
            with tc.tile_pool(name="phF", bufs=1) as phF, \
                 tc.tile_pool(name="ps_F", bufs=4, space="PSUM") as ps_F, \
                 tc.tile_pool(name="ps_tF", bufs=2, space="PSUM") as ps_tF:
                xh2T = [phF.tile([128, TQ], MMDT, tag=f"xh2T{f}",
                                 name=f"xh2T{f}") for f in range(NFC)]
                with tc.tile_pool(name="phF2", bufs=8) as phF2:
                    for rg in range(0, NQT, 4):
                        tiles = []
                        for g in range(4):
                            t_ = phF2.tile([128, C], F32, tag="h2row",
                                           name="h2row")
                            nc.vector.tensor_copy(t_[:], y1[rg + g][:])
                            _layer_norm_inplace(nc, stat_pool, t_, eps_sb)
                            tiles.append(t_)
                        for fc in range(NFC):
                            ps = ps_tF.tile([128, 512], F32, tag="tps",
                                           name="tps")
                            for g in range(4):
                                nc.tensor.transpose(
                                    ps[:, g * 128:(g + 1) * 128],
                                    tiles[g][:, fc * 128:(fc + 1) * 128],
                                    ident[:])
                            nc.vector.tensor_copy(
                                xh2T[fc][:, rg * 128:rg * 128 + 512],
                                ps[:])

                b1_sb = phF.tile([128, FF // 128], F32, tag="b1")
                nc.sync.dma_start(
                    b1_sb[:], b1.rearrange("(m p) o -> p (m o)", p=128))
                b2b_sb = phF.tile([128, C], F32, tag="b2b")
                nc.sync.dma_start(b2b_sb[:], b2b)
                y_acc = [phF.tile([128, C], F32, tag=f"yacc{i}",
                                  name=f"yacc{i}") for i in range(NQT)]
                for qi in range(NQT):
                    nc.vector.tensor_add(y_acc[qi][:], y1[qi][:], b2b_sb[:])
                with tc.tile_pool(name="w1p", bufs=3) as w1p, \
                     tc.tile_pool(name="w2p", bufs=10) as w2p, \
                     tc.tile_pool(name="ffp", bufs=1) as ffp:
                    for stripe in range(4):
                        ff_sb = [ffp.tile([128, TQ], MMDT, tag=f"ff{m}",
                                          name=f"ff{m}") for m in range(8)]
                        w1v = w1.rearrange("(kc p) m -> p kc m", p=128)
                        for m in range(8):
                            mc = stripe * 8 + m
                            w1t = w1p.tile([128, NFC, 128], MMDT, tag="w1t",
                                           name="w1t")
                            nc.sync.dma_start(
                                w1t[:], w1v[:, :, mc * 128:(mc + 1) * 128])
                            w1_sb = [w1t[:, kc, :] for kc in range(NFC)]
                            for nb in range(2):
                                ps = ps_F.tile([128, 512], F32, tag="psqkv",
                                                 name="psf1")
                                for kc in range(NFC):
                                    nc.tensor.matmul(
                                        ps[:],
                                        w1_sb[kc],
                                        r(xh2T[kc][:, nb * 512:
                                                   (nb + 1) * 512]),
                                        start=(kc == 0), stop=(kc == NFC - 1))
                                nc.scalar.activation(
                                    ff_sb[m][:, nb * 512:(nb + 1) * 512],
                                    ps[:], GELU_FUNC,
                                    bias=b1_sb[:, mc:mc + 1])
                        w2_sb = []
                        for m in range(8):
                            mc = stripe * 8 + m
                            wt = w2p.tile([128, C], MMDT, tag="w2t",
                                          name="w2t")
                            nc.sync.dma_start(
                                wt[:], w2[mc * 128:(mc + 1) * 128, :])
                            w2_sb.append(wt)
                        for qi in range(NQT):
                            for jb in range(2):
                                ps = ps_F.tile([128, 512], F32, tag="psqkv",
                                                 name="psf2")
                                for m in range(8):
                                    nc.tensor.matmul(
                                        ps[:],
                                        r(ff_sb[m][:, qi * 128:
                                                   (qi + 1) * 128]),
                                        r(w2_sb[m][:, jb * 512:
                                                   (jb + 1) * 512]),
                                        start=(m == 0), stop=(m == 7))
                                nc.vector.tensor_add(
                                    y_acc[qi][:, jb * 512:(jb + 1) * 512],
                                    y_acc[qi][:, jb * 512:(jb + 1) * 512],
                                    ps[:])
                for qi in range(NQT):
                    nc.sync.dma_start(y_out[qi * 128:(qi + 1) * 128, :],
                                      y_acc[qi][:])

    nc.compile()
    return nc


_NC_CACHE = None


def _get_nc():
    global _NC_CACHE
    if _NC_CACHE is None:
        _NC_CACHE = build_program()
    return _NC_CACHE


def _prep_in_maps(x, Wq, Wk, Wv, Wo, W1, b1, W2, b2, g1, be1, g2, be2):
    x = np.asarray(x, dtype=np.float32)
    Wq = np.asarray(Wq, np.float32); Wk = np.asarray(Wk, np.float32)
    Wv = np.asarray(Wv, np.float32); Wo = np.asarray(Wo, np.float32)
    W1 = np.asarray(W1, np.float32); W2 = np.asarray(W2, np.float32)
    b1 = np.asarray(b1, np.float32); b2 = np.asarray(b2, np.float32)
    g1 = np.asarray(g1, np.float32); be1 = np.asarray(be1, np.float32)
    g2 = np.asarray(g2, np.float32); be2 = np.asarray(be2, np.float32)

    # host-side weight folding (LN affine into projections)
    wqt = np.ascontiguousarray((Wq * g1[None, :]).T)     # [c, j]
    wkt = np.ascontiguousarray((Wk * g1[None, :]).T)
    wvt = np.ascontiguousarray((Wv * g1[None, :]).T)
    wot = np.ascontiguousarray(Wo.T)                      # [f, j]
    w1e = np.ascontiguousarray(W1 * g2[:, None])          # [c, m]
    bq = (be1 @ Wq.T).reshape(C, 1)
    bk = (be1 @ Wk.T).reshape(C, 1)
    bv = (be1 @ Wv.T)                                     # [C]
    bvb = np.ascontiguousarray(np.broadcast_to(bv[None, :], (128, C)))
    b1e = (b1 + be2 @ W1).reshape(FF, 1)
    b2b = np.ascontiguousarray(np.broadcast_to(b2[None, :], (128, C)))

    qaug = np.zeros((2, TQ), np.float32)
    qaug[0, :512] = 1.0
    qaug[1, 512:] = 1.0

    tri = np.zeros((4, 128, 512), np.float32)
    ki = np.arange(128)[:, None]
    qi = np.arange(512)[None, :]
    for rr in range(4):
        tri[rr] = np.where(128 * rr + ki <= qi, 0.0, NEG)

    in_maps = []
    for core in range(8):
        b_, h_ = core // 2, core % 2
        kaug = np.zeros((2, T), np.float32)
        kk = np.arange(T)
        kaug[0, kk >= 1024 * h_ + 512] = NEG
        kaug[1, kk >= 1024 * h_ + 1024] = NEG
        mtri = np.zeros((2, 8, 128, 512), np.float32)
        for c_ in range(2):
            for s in range(8):
                if h_ == 0 and s < 4:
                    mtri[c_, s] = tri[s]
                elif h_ == 1 and s >= 4:
                    mtri[c_, s] = tri[s - 4]
        in_maps.append({
            "xkv": np.ascontiguousarray(x[b_]),
            "xq": np.ascontiguousarray(x[b_, h_ * TQ:(h_ + 1) * TQ]),
            "wqt": wqt.astype(MMNP), "wkt": wkt.astype(MMNP),
            "wvt": wvt.astype(MMNP), "wot": wot.astype(MMNP),
            "w1": w1e.astype(MMNP), "w2": W2.astype(MMNP),
            "bq": bq, "bk": bk, "bvb": bvb, "b1": b1e, "b2b": b2b,
            "kaug": kaug.astype(MMNP), "qaug": qaug.astype(MMNP),
            "mtri": mtri,
        })
    return in_maps


def _gather(res):
    y = np.empty((4, T, C), np.float32)
    for core in range(8):
        b_, h_ = core // 2, core % 2
        y[b_, h_ * TQ:(h_ + 1) * TQ] = res.results[core]["y"]
    return y


def kernel(**inputs):
    in_maps = _prep_in_maps(**inputs)
    nc = _get_nc()
    res = run_bass_kernel_spmd(nc, in_maps, list(range(8)))
    return _gather(res)


def kernel_traced(**inputs):
    import tempfile
    in_maps = _prep_in_maps(**inputs)
    nc = _get_nc()
    res = run_bass_kernel_spmd(nc, in_maps, list(range(8)), trace=True,
                               tmpdir=tempfile.mkdtemp(prefix="ktrace_"))
    return _gather(res), res.exec_time_ns
